# revision 19
# baseline (speedup 1.0000x reference)
"""Trainium2 Bass kernel for nn_LongAttention (gated linear attention).

Full inputs in, full outputs out. 8 NeuronCores, sequence-parallel sharding
(2 batches x 4 chunks of 1024 tokens); [channel -> partition, time -> free]
layout per core.

Perf structure (vs the first working version):
  - All sigmoids are computed as (tanh(z/2)+1)/2 with the 1/2 factors folded
    into host-side params (vn_g/vn_b absorb i_gate's, gn_g/gn_b absorb
    o_gate's), so the whole kernel needs only TWO activation-table sets
    (silu_and_others for conv+gates, sqrt_and_others for all norm stats)
    instead of per-head ln/exp set reloads (~160 loads, ~200us before).
  - rsqrt = reciprocal_approx_fast(sqrt(x)): off the banned Rsqrt ACT and
    off the slow DVE iterative divide.
  - Phase order keeps the PE stream dense (HAM clock gate stays warm):
    conv -> gamma/i/o gate projections -> per-head q/k/v + scan pipeline ->
    carry collective -> batched epilogue -> Wo.
  - Per-(head,time) stats go through one-hot stationary colsum matmuls into
    shared [16,T] PSUM tiles; epilogue row math runs batched across heads.
  - Cross-chunk carry fixup (G_t * s) is a PE outer product.
"""

import numpy as np

B, T, C, H, K = 2, 4096, 2048, 16, 4
D = C // H
EPS = 1e-5
N_CORES = 8
TC = (B * T) // N_CORES          # tokens per core (1024)
NCT = C // 128                   # channel tiles (16)
HALO = K - 1                     # conv halo (3)

_cache = {}


def _build_program():
    import concourse.bacc as bacc
    import concourse.mybir as mybir
    import concourse.tile as tile

    dt = mybir.dt
    AF = mybir.ActivationFunctionType
    OP = mybir.AluOpType

    nc = bacc.Bacc("TRN2", target_bir_lowering=False, debug=False,
                   num_devices=N_CORES)

    # ---- external inputs (per core) ----
    xT = nc.dram_tensor("xT", [C, TC + HALO], dt.bfloat16, kind="ExternalInput")
    wq = nc.dram_tensor("wq", [NCT, 128, NCT, 128], dt.bfloat16, kind="ExternalInput")
    wk = nc.dram_tensor("wk", [NCT, 128, NCT, 128], dt.bfloat16, kind="ExternalInput")
    wv = nc.dram_tensor("wv", [NCT, 128, NCT, 128], dt.bfloat16, kind="ExternalInput")
    wig = nc.dram_tensor("wig", [NCT, 128, NCT, 128], dt.bfloat16, kind="ExternalInput")
    wog = nc.dram_tensor("wog", [NCT, 128, NCT, 128], dt.bfloat16, kind="ExternalInput")
    wo = nc.dram_tensor("wo", [NCT, 128, NCT, 128], dt.bfloat16, kind="ExternalInput")
    wgam = nc.dram_tensor("wgam", [NCT, 128, H], dt.bfloat16, kind="ExternalInput")
    convw = nc.dram_tensor("convw", [NCT, 128, K], dt.float32, kind="ExternalInput")
    convb = nc.dram_tensor("convb", [128, NCT], dt.float32, kind="ExternalInput")
    bigc = nc.dram_tensor("bigc", [128, NCT], dt.float32, kind="ExternalInput")   # big/2
    bogc = nc.dram_tensor("bogc", [128, NCT], dt.float32, kind="ExternalInput")   # bog/2
    gngc = nc.dram_tensor("gngc", [128, NCT], dt.float32, kind="ExternalInput")   # gn_g/2
    gnbc = nc.dram_tensor("gnbc", [128, NCT], dt.float32, kind="ExternalInput")   # gn_b/2
    vng = nc.dram_tensor("vng", [128, 1], dt.float32, kind="ExternalInput")       # vn_g/2
    vnb = nc.dram_tensor("vnb", [128, 1], dt.float32, kind="ExternalInput")       # vn_b/2
    mng = nc.dram_tensor("mng", [128, 1], dt.float32, kind="ExternalInput")
    mnb = nc.dram_tensor("mnb", [128, 1], dt.float32, kind="ExternalInput")
    bgam = nc.dram_tensor("bgam", [H, 1], dt.float32, kind="ExternalInput")       # bgam/2
    sel = nc.dram_tensor("sel", [H, N_CORES], dt.float32, kind="ExternalInput")
    yT = nc.dram_tensor("yT", [C, TC], dt.float32, kind="ExternalOutput")

    SLAB = 512
    NH = TC // SLAB  # matmul slabs per row (2)

    with tile.TileContext(nc) as tc:
        with tc.tile_pool(name="big", bufs=1) as big, \
             tc.tile_pool(name="wp", bufs=2) as wp, \
             tc.tile_pool(name="rawp", bufs=8) as rawp, \
             tc.tile_pool(name="f32p", bufs=2) as f32p, \
             tc.tile_pool(name="gbp", bufs=2) as gbp, \
             tc.tile_pool(name="bcp", bufs=2) as bcp, \
             tc.tile_pool(name="rowp", bufs=2) as rowp, \
             tc.tile_pool(name="r16p", bufs=3) as r16p, \
             tc.tile_pool(name="ccp", bufs=5) as ccp, \
             tc.tile_pool(name="pp", bufs=2, space="PSUM") as pp, \
             tc.tile_pool(name="spp", bufs=2, space="PSUM") as spp, \
             tc.tile_pool(name="dram", bufs=1, space="DRAM") as dram:

            # ---- persistent SBUF tiles ----
            xts = big.tile([128, NCT, TC + HALO], dt.bfloat16, tag="bigA")
            xc = big.tile([128, NCT, TC], dt.bfloat16, tag="bigB")
            cwsb = big.tile([128, NCT, K], dt.float32)
            convb_sb = big.tile([128, NCT], dt.float32)
            bigc_sb = big.tile([128, NCT], dt.float32)
            bogc_sb = big.tile([128, NCT], dt.float32)
            gngc_sb = big.tile([128, NCT], dt.float32)
            gnbc_sb = big.tile([128, NCT], dt.float32)
            vng_sb = big.tile([128, 1], dt.float32)
            vnb_sb = big.tile([128, 1], dt.float32)
            mng_sb = big.tile([128, 1], dt.float32)
            mnb_sb = big.tile([128, 1], dt.float32)
            bgam_sb = big.tile([H, 1], dt.float32)
            sel_sb = big.tile([H, N_CORES], dt.float32)
            wgam_sb = big.tile([128, NCT, H], dt.bfloat16)
            ones16 = big.tile([H, TC], dt.bfloat16)
            srow_bf = big.tile([1, C], dt.bfloat16)
            onesr = big.tile([1, 128], dt.bfloat16)
            acol16 = big.tile([128, H], dt.float32)
            g_last = big.tile([1, H], dt.float32)
            rm_bf = big.tile([H, 2 * TC], dt.bfloat16)
            grm_bf = big.tile([H, 2 * TC], dt.bfloat16)
            # one-hot stationaries for colsum matmuls
            oh4 = [big.tile([128, 4], dt.bfloat16, name=f"oh4_{r}")
                   for r in range(4)]
            oh16 = [big.tile([128, H], dt.bfloat16, name=f"oh16_{r}")
                    for r in range(H)]
            eps1 = big.tile([1, 1], dt.float32)
            eps16 = big.tile([H, 1], dt.float32)

            nc.gpsimd.memset(eps1[:], EPS)
            nc.gpsimd.memset(eps16[:], EPS)
            nc.gpsimd.memset(ones16[:], 1.0)
            nc.gpsimd.memset(onesr[:], 1.0)
            for r in range(4):
                nc.gpsimd.memset(oh4[r][:], 0.0)
                nc.gpsimd.memset(oh4[r][:, r:r + 1], 1.0)
            for r in range(H):
                nc.gpsimd.memset(oh16[r][:], 0.0)
                nc.gpsimd.memset(oh16[r][:, r:r + 1], 1.0)

            # DRAM scratch + collective bounce buffers
            qn_dram = dram.tile([NCT, 128, TC], dt.bfloat16)
            ig_dram = dram.tile([NCT, 128, TC], dt.bfloat16)
            og_dram = dram.tile([NCT, 128, TC], dt.bfloat16)
            mem_dram = dram.tile([NCT, 128, TC], dt.bfloat16)
            gamma_dram = dram.tile([H, TC], dt.float32)
            G_dram = dram.tile([H, TC], dt.float32)
            G_bf_dram = dram.tile([H, TC], dt.bfloat16)
            cc_in = dram.tile([1, 2 * C], dt.float32)
            cc_out = dram.tile([N_CORES, 2 * C], dt.float32, addr_space="Shared")
            s_dram = dram.tile([H, 128], dt.bfloat16)

            # ---- load constants + x^T ----
            for i in range(NCT):
                nc.sync.dma_start(xts[:, i, :], xT[i * 128:(i + 1) * 128, :])
                nc.sync.dma_start(cwsb[:, i, :], convw[i])
                nc.sync.dma_start(wgam_sb[:, i, :], wgam[i])
            nc.sync.dma_start(convb_sb[:], convb[:])
            nc.sync.dma_start(bigc_sb[:], bigc[:])
            nc.sync.dma_start(bogc_sb[:], bogc[:])
            nc.sync.dma_start(gngc_sb[:], gngc[:])
            nc.sync.dma_start(gnbc_sb[:], gnbc[:])
            nc.sync.dma_start(vng_sb[:], vng[:])
            nc.sync.dma_start(vnb_sb[:], vnb[:])
            nc.sync.dma_start(mng_sb[:], mng[:])
            nc.sync.dma_start(mnb_sb[:], mnb[:])
            nc.sync.dma_start(bgam_sb[:], bgam[:])
            nc.sync.dma_start(sel_sb[:], sel[:])

            # ---- phase S0: causal depthwise conv + SiLU -> xc (bf16) ----
            for i in range(NCT):
                a0 = rawp.tile([128, TC], dt.bfloat16, tag="cacc", bufs=2)
                a1 = rawp.tile([128, TC], dt.bfloat16, tag="cacc", bufs=2)
                nc.vector.tensor_scalar_mul(a0[:], xts[:, i, 0:TC], cwsb[:, i, 0:1])
                nc.vector.scalar_tensor_tensor(a1[:], xts[:, i, 1:1 + TC],
                                               cwsb[:, i, 1:2], a0[:],
                                               OP.mult, OP.add)
                nc.vector.scalar_tensor_tensor(a0[:], xts[:, i, 2:2 + TC],
                                               cwsb[:, i, 2:3], a1[:],
                                               OP.mult, OP.add)
                nc.vector.scalar_tensor_tensor(a1[:], xts[:, i, 3:3 + TC],
                                               cwsb[:, i, 3:4], a0[:],
                                               OP.mult, OP.add)
                nc.scalar.activation(xc[:, i, :], a1[:], AF.Silu,
                                     bias=convb_sb[:, i:i + 1], scale=1.0)

            def proj_psum(wtensor, h, rhs_view, tag="pp"):
                ps = pp.tile([128, TC], dt.float32, tag=tag)
                wt = wp.tile([128, NCT, 128], dt.bfloat16, tag="wt")
                nc.sync.dma_start(wt[:], wtensor[h])
                for k in range(NCT):
                    for s in range(NH):
                        nc.tensor.matmul(ps[:, s * SLAB:(s + 1) * SLAB],
                                         wt[:, k, :], rhs_view(k, s),
                                         start=(k == 0), stop=(k == NCT - 1))
                return ps

            def warm_pe(n):
                # dependency-free matmuls that keep the PE HAM clock gate
                # warm across windows where real PE work is blocked
                wtile = pp.tile([128, SLAB], dt.float32, tag="pp")
                for _ in range(n):
                    nc.tensor.matmul(wtile[:], onesr[:], ones16[0:1, 0:SLAB],
                                     start=True, stop=True)

            def x_rhs(k, s):
                return xts[:, k, HALO + s * SLAB:HALO + (s + 1) * SLAB]

            def xc_rhs(k, s):
                return xc[:, k, s * SLAB:(s + 1) * SLAB]

            # ---- phase S1a: gamma = (tanh(glog/2+bgam/2)+1)/2 ; G cumprod ----
            glog = spp.tile([H, TC], dt.float32, tag="spt")
            for k in range(NCT):
                for s in range(NH):
                    nc.tensor.matmul(glog[:, s * SLAB:(s + 1) * SLAB],
                                     wgam_sb[:, k, :],
                                     xc[:, k, s * SLAB:(s + 1) * SLAB],
                                     start=(k == 0), stop=(k == NCT - 1))
            tgam = r16p.tile([H, TC], dt.float32, tag="r16")
            nc.scalar.activation(tgam[:], glog[:], AF.Tanh,
                                 bias=bgam_sb[:], scale=0.5)
            gamma_t = r16p.tile([H, TC], dt.float32, tag="r16")
            nc.vector.tensor_scalar(gamma_t[:], tgam[:], 0.5, 0.5,
                                    OP.mult, OP.add)
            nc.sync.dma_start(gamma_dram[:], gamma_t[:])
            G_t = r16p.tile([H, TC], dt.float32, tag="r16")
            nc.vector.tensor_tensor_scan(G_t[:], gamma_t[:], ones16[:], 1.0,
                                         OP.mult, OP.mult)
            nc.sync.dma_start(G_dram[:], G_t[:])
            G_bft = r16p.tile([H, TC], dt.bfloat16, tag="r16")
            nc.vector.tensor_copy(G_bft[:], G_t[:])
            nc.sync.dma_start(G_bf_dram[:], G_bft[:])
            # batched A-carry: cc_in[0, 0:C] = repeat(G[:, -1], 128)
            nc.sync.dma_start(g_last[:], G_dram[:, TC - 1:TC])
            nc.gpsimd.partition_broadcast(acol16[:], g_last[:])
            for hh in range(H):
                nc.scalar.dma_start(cc_in[0:1, hh * 128:(hh + 1) * 128],
                                    acol16[:, hh:hh + 1])

            # ---- phase S1b: i/o gates as tanh -> DRAM (ACT set: silu/tanh) ----
            for h in range(H):
                psI = proj_psum(wig, h, xc_rhs)
                igt = rawp.tile([128, TC], dt.bfloat16, tag="chn", bufs=5)
                nc.scalar.activation(igt[:], psI[:], AF.Tanh,
                                     bias=bigc_sb[:, h:h + 1], scale=0.5)
                nc.scalar.dma_start(ig_dram[h], igt[:])

            # ---- phase S2: per-head q/k/v proj + norms + kv + scan ----
            # Pipelined so the PE stream stays dense: head h-1's colsums and
            # stats chain are emitted between head h's projection blocks.
            def emit_one_proj(wtensor, h, nm):
                ps = proj_psum(wtensor, h, x_rhs)
                raw = rawp.tile([128, TC], dt.bfloat16, tag="kvq", bufs=7,
                                name=f"{nm}raw{h}")
                nc.scalar.copy(raw[:], ps[:])
                sq = rawp.tile([128, TC], dt.bfloat16, tag="sq", bufs=4,
                               name=f"{nm}sq{h}")
                nc.vector.tensor_mul(sq[:], raw[:], raw[:])
                return raw, sq

            def colsum(src_bf16):
                """sum over partitions via ones-column matmul -> [1,TC] psum"""
                row = spp.tile([1, TC], dt.float32, tag="spt")
                for s in range(NH):
                    sl = slice(s * SLAB, (s + 1) * SLAB)
                    nc.tensor.matmul(row[:, sl], oh4[0][:, 0:1], src_bf16[:, sl],
                                     start=True, stop=True)
                return row

            def rsqrt_row(ssq_psum):
                """[1,TC] psum -> [1,TC] bf16 rsqrt via Sqrt + recip_fast"""
                s0 = rowp.tile([1, TC], dt.float32, tag="rf", bufs=3)
                nc.scalar.activation(s0[:], ssq_psum[:], AF.Sqrt,
                                     bias=0.0, scale=1.0)
                r0 = rowp.tile([1, TC], dt.float32, tag="rf", bufs=3)
                nc.vector.reciprocal_approx_fast(r0[:], s0[:])
                rb = rowp.tile([1, TC], dt.bfloat16, tag="rb", bufs=3)
                nc.vector.tensor_copy(rb[:], r0[:])
                return rb

            def emit_chain(h, kraw, vraw, qraw, ksq, vsq, qsq):
                # k/q l2 rsqrt rows (partition 0) + v LN rows
                kssq = colsum(ksq)
                rk_bf = rsqrt_row(kssq)
                rkb = bcp.tile([128, TC], dt.bfloat16, tag="bc1", bufs=4)
                nc.gpsimd.partition_broadcast(rkb[:], rk_bf[:])
                qssq = colsum(qsq)
                rq_bf = rsqrt_row(qssq)
                rqb = bcp.tile([128, TC], dt.bfloat16, tag="bc1", bufs=4)
                nc.gpsimd.partition_broadcast(rqb[:], rq_bf[:])

                vsum = colsum(vraw)
                mean = rowp.tile([1, TC], dt.float32, tag="rf", bufs=3)
                nc.vector.tensor_scalar_mul(mean[:], vsum[:], 1.0 / D)
                vssq = colsum(vsq)
                msq = rowp.tile([1, TC], dt.float32, tag="rf", bufs=3)
                nc.vector.tensor_mul(msq[:], mean[:], mean[:])
                var = rowp.tile([1, TC], dt.float32, tag="rf", bufs=3)
                nc.vector.scalar_tensor_tensor(var[:], vssq[:], 1.0 / D,
                                               msq[:], OP.mult, OP.subtract)
                sv = rowp.tile([1, TC], dt.float32, tag="rf", bufs=3)
                nc.scalar.activation(sv[:], var[:], AF.Sqrt,
                                     bias=eps1[:], scale=1.0)
                rv = rowp.tile([1, TC], dt.float32, tag="rf", bufs=3)
                nc.vector.reciprocal_approx_fast(rv[:], sv[:])
                rv_bf = rowp.tile([1, TC], dt.bfloat16, tag="rb", bufs=3)
                nc.vector.tensor_copy(rv_bf[:], rv[:])
                mv_bf = rowp.tile([1, TC], dt.bfloat16, tag="rb", bufs=3)
                nc.vector.tensor_mul(mv_bf[:], mean[:], rv[:])
                rvb = bcp.tile([128, TC], dt.bfloat16, tag="bc1", bufs=4)
                nc.gpsimd.partition_broadcast(rvb[:], rv_bf[:])
                mvb = bcp.tile([128, TC], dt.bfloat16, tag="bc1", bufs=4)
                nc.gpsimd.partition_broadcast(mvb[:], mv_bf[:])

                kn = rawp.tile([128, TC], dt.bfloat16, tag="chn", bufs=5)
                nc.vector.tensor_mul(kn[:], kraw[:], rkb[:])
                v1 = rawp.tile([128, TC], dt.bfloat16, tag="chn", bufs=5)
                nc.vector.tensor_mul(v1[:], vraw[:], rvb[:])
                v2 = rawp.tile([128, TC], dt.bfloat16, tag="chn", bufs=5)
                nc.vector.tensor_sub(v2[:], v1[:], mvb[:])
                vn = rawp.tile([128, TC], dt.bfloat16, tag="chn", bufs=5)
                nc.scalar.activation(vn[:], v2[:], AF.Identity,
                                     bias=vnb_sb[:], scale=vng_sb[:])
                qn = rawp.tile([128, TC], dt.bfloat16, tag="chn", bufs=5)
                nc.vector.tensor_mul(qn[:], qraw[:], rqb[:])
                nc.gpsimd.dma_start(qn_dram[h], qn[:])

                igt = rawp.tile([128, TC], dt.bfloat16, tag="chn", bufs=5)
                nc.sync.dma_start(igt[:], ig_dram[h])
                kv1 = rawp.tile([128, TC], dt.bfloat16, tag="chn", bufs=5)
                nc.vector.tensor_mul(kv1[:], kn[:], vn[:])
                kvh = f32p.tile([128, TC], dt.float32, tag="f32t")
                nc.vector.scalar_tensor_tensor(kvh[:], igt[:], 1.0, kv1[:],
                                               OP.add, OP.mult)

                gam0 = rowp.tile([1, TC], dt.float32, tag="rf", bufs=3)
                nc.sync.dma_start(gam0[:], gamma_dram[h:h + 1, :])
                gb = gbp.tile([128, TC], dt.float32, tag="gb")
                nc.gpsimd.partition_broadcast(gb[:], gam0[:])
                smem = f32p.tile([128, TC], dt.float32, tag="f32t")
                nc.vector.tensor_tensor_scan(smem[:], gb[:], kvh[:], 0.0,
                                             OP.mult, OP.add)
                memb = rawp.tile([128, TC], dt.bfloat16, tag="chn", bufs=5)
                nc.scalar.copy(memb[:], smem[:])
                nc.scalar.dma_start(mem_dram[h], memb[:])

                # carry B = final state column (A was batched after S1a)
                nc.gpsimd.dma_start(cc_in[0:1, C + h * 128:C + (h + 1) * 128],
                                     smem[:, TC - 1:TC])

            prev = None
            for h in range(H):
                kraw, ksq = emit_one_proj(wk, h, "k")
                vraw, vsq = emit_one_proj(wv, h, "v")
                if prev is not None:
                    emit_chain(*prev)
                qraw, qsq = emit_one_proj(wq, h, "q")
                prev = (h, kraw, vraw, qraw, ksq, vsq, qsq)
            emit_chain(*prev)

            # ---- o-gate projections here: PE work that hides the
            # collective + combine latency (og only consumed in S4e) ----
            for h in range(H):
                psO = proj_psum(wog, h, xc_rhs)
                ogt = rawp.tile([128, TC], dt.bfloat16, tag="chn", bufs=5)
                nc.scalar.activation(ogt[:], psO[:], AF.Tanh,
                                     bias=bogc_sb[:, h:h + 1], scale=0.5)
                nc.scalar.dma_start(og_dram[h], ogt[:])

            # ---- phase S3: all-gather carries + prefix combine ----
            warm_pe(300)
            nc.gpsimd.collective_compute(
                "AllGather", OP.bypass,
                replica_groups=[list(range(N_CORES))],
                ins=[cc_in[:]], outs=[cc_out[:]],
            )
            srun = ccp.tile([H, 128], dt.float32, tag="cc")
            ssel = ccp.tile([H, 128], dt.float32, tag="cc")
            nc.gpsimd.memset(srun[:], 0.0)
            nc.gpsimd.memset(ssel[:], 0.0)
            for j in range(N_CORES):
                ssel2 = ccp.tile([H, 128], dt.float32, tag="cc")
                nc.vector.scalar_tensor_tensor(ssel2[:], srun[:],
                                               sel_sb[:, j:j + 1], ssel[:],
                                               OP.mult, OP.add)
                ssel = ssel2
                if j == N_CORES - 1:
                    break
                if j == (N_CORES // 2) - 1:
                    srun = ccp.tile([H, 128], dt.float32, tag="cc")
                    nc.gpsimd.memset(srun[:], 0.0)
                else:
                    arow = ccp.tile([H, 128], dt.float32, tag="cc")
                    brow = ccp.tile([H, 128], dt.float32, tag="cc")
                    nc.sync.dma_start(arow[:], cc_out[j:j + 1, 0:C])
                    nc.sync.dma_start(brow[:], cc_out[j:j + 1, C:2 * C])
                    tmp = ccp.tile([H, 128], dt.float32, tag="cc")
                    nc.vector.tensor_mul(tmp[:], arow[:], srun[:])
                    nc.vector.tensor_add(tmp[:], tmp[:], brow[:])
                    srun = tmp
            ssel_bf = ccp.tile([H, 128], dt.bfloat16, tag="ccb", bufs=1)
            nc.vector.tensor_copy(ssel_bf[:], ssel[:])
            nc.sync.dma_start(s_dram[:], ssel_bf[:])
            for h in range(H):
                nc.sync.dma_start(srow_bf[0:1, h * 128:(h + 1) * 128],
                                  s_dram[h:h + 1, :])

            # ---- phase S4: epilogue (fixup + LN + *q + GroupNorm + *og) ----
            memfall = big.tile([128, NCT, TC], dt.bfloat16, tag="bigB")
            Msum = spp.tile([H, TC], dt.float32, tag="spt")
            Msq = spp.tile([H, TC], dt.float32, tag="spt")

            # S4a: fixup memf = outer(s, G) + memb, plus LN colsums
            for h in range(H):
                membh = rawp.tile([128, TC], dt.bfloat16, tag="chn", bufs=5)
                nc.sync.dma_start(membh[:], mem_dram[h])
                g0bf = rowp.tile([1, TC], dt.bfloat16, tag="rb", bufs=3)
                nc.sync.dma_start(g0bf[:], G_bf_dram[h:h + 1, :])
                ps_sG = pp.tile([128, TC], dt.float32, tag="pp")
                for s in range(NH):
                    sl = slice(s * SLAB, (s + 1) * SLAB)
                    nc.tensor.matmul(ps_sG[:, sl],
                                     srow_bf[0:1, h * 128:(h + 1) * 128],
                                     g0bf[0:1, sl], start=True, stop=True)
                nc.vector.tensor_add(memfall[:, h, :], ps_sG[:], membh[:])
                mfsq = rawp.tile([128, TC], dt.bfloat16, tag="chn", bufs=5)
                nc.vector.tensor_mul(mfsq[:], memfall[:, h, :], memfall[:, h, :])
                for s in range(NH):
                    sl = slice(s * SLAB, (s + 1) * SLAB)
                    nc.tensor.matmul(Msum[:, sl], oh16[h][:],
                                     memfall[:, h, sl],
                                     start=(h == 0), stop=(h == H - 1))
                    nc.tensor.matmul(Msq[:, sl], oh16[h][:], mfsq[:, sl],
                                     start=(h == 0), stop=(h == H - 1))

            # S4b: batched LN row stats across all heads
            def batched_rows(SumT, SqT, out_bf):
                mean16 = r16p.tile([H, TC], dt.float32, tag="r16")
                nc.vector.tensor_scalar_mul(mean16[:], SumT[:], 1.0 / D)
                msq16 = r16p.tile([H, TC], dt.float32, tag="r16")
                nc.vector.tensor_mul(msq16[:], mean16[:], mean16[:])
                var16 = r16p.tile([H, TC], dt.float32, tag="r16")
                nc.vector.scalar_tensor_tensor(var16[:], SqT[:], 1.0 / D,
                                               msq16[:], OP.mult, OP.subtract)
                sv16 = r16p.tile([H, TC], dt.float32, tag="r16")
                nc.scalar.activation(sv16[:], var16[:], AF.Sqrt,
                                     bias=eps16[:], scale=1.0)
                rstd16 = r16p.tile([H, TC], dt.float32, tag="r16")
                nc.vector.reciprocal_approx_fast(rstd16[:], sv16[:])
                nc.vector.tensor_copy(out_bf[:, 0:TC], rstd16[:])
                nc.vector.tensor_mul(out_bf[:, TC:2 * TC], mean16[:], rstd16[:])

            warm_pe(40)
            batched_rows(Msum, Msq, rm_bf)

            # S4c: normalize mem, multiply by q, GN colsums
            Gsum = spp.tile([H, TC], dt.float32, tag="spt")
            Gsq = spp.tile([H, TC], dt.float32, tag="spt")
            for h in range(H):
                rmst = rowp.tile([1, 2 * TC], dt.bfloat16, tag="st2", bufs=2)
                nc.sync.dma_start(rmst[:], rm_bf[h:h + 1, :])
                ps_r = pp.tile([128, TC], dt.float32, tag="pp")
                ps_m = pp.tile([128, TC], dt.float32, tag="pp")
                for s in range(NH):
                    sl = slice(s * SLAB, (s + 1) * SLAB)
                    nc.tensor.matmul(ps_r[:, sl], onesr[:],
                                     rmst[0:1, s * SLAB:(s + 1) * SLAB],
                                     start=True, stop=True)
                    nc.tensor.matmul(ps_m[:, sl], onesr[:],
                                     rmst[0:1, TC + s * SLAB:TC + (s + 1) * SLAB],
                                     start=True, stop=True)
                m1 = rawp.tile([128, TC], dt.bfloat16, tag="chn", bufs=5)
                nc.vector.tensor_mul(m1[:], memfall[:, h, :], ps_r[:])
                m2 = rawp.tile([128, TC], dt.bfloat16, tag="chn", bufs=5)
                nc.vector.tensor_sub(m2[:], m1[:], ps_m[:])
                memn = rawp.tile([128, TC], dt.bfloat16, tag="chn", bufs=5)
                nc.scalar.activation(memn[:], m2[:], AF.Identity,
                                     bias=mnb_sb[:], scale=mng_sb[:])
                qnh = rawp.tile([128, TC], dt.bfloat16, tag="chn", bufs=5)
                nc.sync.dma_start(qnh[:], qn_dram[h])
                nc.vector.tensor_mul(memfall[:, h, :], memn[:], qnh[:])
                psq = rawp.tile([128, TC], dt.bfloat16, tag="chn", bufs=5)
                nc.vector.tensor_mul(psq[:], memfall[:, h, :], memfall[:, h, :])
                for s in range(NH):
                    sl = slice(s * SLAB, (s + 1) * SLAB)
                    nc.tensor.matmul(Gsum[:, sl], oh16[h][:],
                                     memfall[:, h, sl],
                                     start=(h == 0), stop=(h == H - 1))
                    nc.tensor.matmul(Gsq[:, sl], oh16[h][:], psq[:, sl],
                                     start=(h == 0), stop=(h == H - 1))

            # S4d: batched GN row stats
            warm_pe(40)
            batched_rows(Gsum, Gsq, grm_bf)

            # S4e: apply GN affine (gn/2 folded) and out gate (tanh+1)
            outsb = big.tile([128, NCT, TC], dt.bfloat16, tag="bigA")
            for h in range(H):
                grst = rowp.tile([1, 2 * TC], dt.bfloat16, tag="st2", bufs=2)
                nc.sync.dma_start(grst[:], grm_bf[h:h + 1, :])
                ps_r = pp.tile([128, TC], dt.float32, tag="pp")
                ps_m = pp.tile([128, TC], dt.float32, tag="pp")
                for s in range(NH):
                    sl = slice(s * SLAB, (s + 1) * SLAB)
                    nc.tensor.matmul(ps_r[:, sl], onesr[:],
                                     grst[0:1, s * SLAB:(s + 1) * SLAB],
                                     start=True, stop=True)
                    nc.tensor.matmul(ps_m[:, sl], onesr[:],
                                     grst[0:1, TC + s * SLAB:TC + (s + 1) * SLAB],
                                     start=True, stop=True)
                g1 = rawp.tile([128, TC], dt.bfloat16, tag="chn", bufs=5)
                nc.vector.tensor_mul(g1[:], memfall[:, h, :], ps_r[:])
                g2 = rawp.tile([128, TC], dt.bfloat16, tag="chn", bufs=5)
                nc.vector.tensor_sub(g2[:], g1[:], ps_m[:])
                outn = rawp.tile([128, TC], dt.bfloat16, tag="chn", bufs=5)
                nc.scalar.activation(outn[:], g2[:], AF.Identity,
                                     bias=gnbc_sb[:, h:h + 1],
                                     scale=gngc_sb[:, h:h + 1])
                ogt = rawp.tile([128, TC], dt.bfloat16, tag="chn", bufs=5)
                nc.sync.dma_start(ogt[:], og_dram[h])
                nc.vector.scalar_tensor_tensor(outsb[:, h, :], ogt[:], 1.0,
                                               outn[:], OP.add, OP.mult)

            # ---- phase S5: Wo matmul -> yT ----
            for oc in range(NCT):
                ps = pp.tile([128, TC], dt.float32, tag="pp")
                wt = wp.tile([128, NCT, 128], dt.bfloat16, tag="wt")
                nc.sync.dma_start(wt[:], wo[oc])
                for k in range(NCT):
                    for s in range(NH):
                        nc.tensor.matmul(ps[:, s * SLAB:(s + 1) * SLAB],
                                         wt[:, k, :],
                                         outsb[:, k, s * SLAB:(s + 1) * SLAB],
                                         start=(k == 0), stop=(k == NCT - 1))
                ysb = rawp.tile([128, TC], dt.float32, tag="ysb", bufs=2)
                nc.scalar.copy(ysb[:], ps[:])
                nc.scalar.dma_start(yT[oc * 128:(oc + 1) * 128, :], ysb[:])

    nc.finalize()
    return nc


def _host_prep(inputs):
    import ml_dtypes
    bf16 = ml_dtypes.bfloat16

    def wtiles(w):
        # per-head stationary block [oc, p, k, c]: w_sb[p, k, c] = W.T[k*128+p, oc*128+c]
        wT = np.ascontiguousarray(w.T.astype(bf16))          # [C_in, C_out]
        return np.ascontiguousarray(
            wT.reshape(NCT, 128, NCT, 128).transpose(2, 1, 0, 3))

    def colmat(v, scale=1.0):
        return np.ascontiguousarray(
            (np.asarray(v, np.float32) * scale).reshape(NCT, 128).T)

    x = np.asarray(inputs["x"])
    common = dict(
        wq=wtiles(np.asarray(inputs["Wq"])),
        wk=wtiles(np.asarray(inputs["Wk"])),
        wv=wtiles(np.asarray(inputs["Wv"])),
        wig=wtiles(np.asarray(inputs["Wig"])),
        wog=wtiles(np.asarray(inputs["Wog"])),
        wo=wtiles(np.asarray(inputs["Wo"])),
        wgam=np.ascontiguousarray(
            np.asarray(inputs["Wgam"]).T.astype(bf16).reshape(NCT, 128, H)),
        convw=np.ascontiguousarray(
            np.asarray(inputs["conv_w"])[:, 0, :].astype(np.float32).reshape(NCT, 128, K)),
        convb=colmat(inputs["conv_b"]),
        bigc=colmat(inputs["big"], 0.5),
        bogc=colmat(inputs["bog"], 0.5),
        gngc=colmat(inputs["gn_g"], 0.5),
        gnbc=colmat(inputs["gn_b"], 0.5),
        vng=np.asarray(inputs["vn_g"], np.float32).reshape(128, 1) * 0.5,
        vnb=np.asarray(inputs["vn_b"], np.float32).reshape(128, 1) * 0.5,
        mng=np.asarray(inputs["mn_g"], np.float32).reshape(128, 1),
        mnb=np.asarray(inputs["mn_b"], np.float32).reshape(128, 1),
        bgam=np.asarray(inputs["bgam"], np.float32).reshape(H, 1) * 0.5,
    )

    xbf = x.astype(bf16)
    in_maps = []
    for core in range(N_CORES):
        b, j = divmod(core, N_CORES // B)
        lo = j * TC
        chunk = np.zeros((TC + HALO, C), bf16)
        src_lo = max(0, lo - HALO)
        chunk[HALO - (lo - src_lo):] = xbf[b, src_lo:lo + TC]
        selv = np.zeros((H, N_CORES), np.float32)
        selv[:, core] = 1.0
        m = dict(common)
        m["xT"] = np.ascontiguousarray(chunk.T)
        m["sel"] = selv
        in_maps.append(m)
    return in_maps


def kernel(**inputs):
    from concourse.bass_utils import run_bass_kernel_spmd

    if "nc" not in _cache:
        _cache["nc"] = _build_program()
    nc = _cache["nc"]

    in_maps = _host_prep(inputs)
    res = run_bass_kernel_spmd(nc, in_maps, core_ids=list(range(N_CORES)))

    y = np.empty((B, T, C), np.float32)
    for core in range(N_CORES):
        b, j = divmod(core, N_CORES // B)
        y[b, j * TC:(j + 1) * TC] = res.results[core]["yT"].T
    return y
